# revision 2
# baseline (speedup 1.0000x reference)
"""Trainium2 Bass kernel for nn_EqAMPBC (FWM/XPM nonlinear equalizer), v3.

Data parallel over 8 cores (131072 -> 16384/core), 32 chunks of N=512 in a
transposed layout (features on partitions, samples on free dim).

v3 structure:
  - Conjugate symmetry S[(-m, m+n)] = conj(S[(m,n)]) cuts FWM product rows
    from 520 to 374 (3 sub-chunks); w2 tables fold members back with sign
    flips.
  - The FWM product operands are pre-gathered on the host (im2col-style
    layout transform, no arithmetic): XA = [a_r|a_i], XCp = [c_r|c_i] row
    stacks. Products are then two big all-SBUF bf16 DVE ops per chunk
    (FD=3072, 2x mode) -- no PE gathers, no PSUM staging, no copies.
  - q-stage products (t*x) and conv2 cross-mode products are packed into
    four [123, N] stacked tiles multiplied on Pool; stacked operands come
    from host-packed DRAM plus the on-device w2 str/sti copies.
  - |x|^2 is one DVE square per chunk; s1+s2 folds into the reduction
    matmuls. PSUM: pAs(x2), ptr, pti, pF/pG = 5 banks; the As->w2->pF->
    transpose chain pipelines across chunks with no bank serialization.
"""
import sys
import numpy as np

sys.path.insert(0, "/opt/trn_rl_repo")

M = 41
P = 20
RHO = 1.0
NCORES = 8
N = 512
F_ROWS = 8


def _fwm_index():
    h = M // 2
    ms, ns = [], []
    for m in range(-h, h + 1):
        for n in range(-h, h + 1):
            if m != 0 and n != 0 and abs(m * n) <= RHO * h and abs(m + n) <= h:
                ms.append(m)
                ns.append(n)
    return np.asarray(ms, np.int32), np.asarray(ns, np.int32)


M_IDX, N_IDX = _fwm_index()
HDIM = len(M_IDX)

# conjugate-pair reps: S[(-m, m+n)] = conj(S[(m,n)])
_idx = {(int(m), int(n)): i for i, (m, n) in enumerate(zip(M_IDX, N_IDX))}
REP_OF = {}
REPS = []
for _i, (_m, _n) in enumerate(zip(M_IDX.tolist(), N_IDX.tolist())):
    if _i in REP_OF:
        continue
    _pos = len(REPS)
    REPS.append(_i)
    REP_OF[_i] = (_pos, False)
    _j = _idx.get((-_m, _m + _n))
    if _j is not None and _j != _i and _j not in REP_OF:
        REP_OF[_j] = (_pos, True)
REPS = np.asarray(REPS, np.int32)
NREP = len(REPS)          # 187
ROWS = 2 * NREP           # 374
A_TAP = P + N_IDX[REPS]
C_TAP = P + M_IDX[REPS] + N_IDX[REPS]
SUBCH = [(0, 128), (128, 128), (256, ROWS - 256)]

_G = np.arange(ROWS)
SRC_A = (_G % 2) * 41 + A_TAP[_G // 2]    # input row for a-side, per product row
SRC_C = (_G % 2) * 41 + C_TAP[_G // 2]


def _build_tables(fwm_wr, fwm_wi, conv1_w, conv2_w, C00):
    t = {}
    for u, (o, ln) in enumerate(SUBCH):
        loc = np.arange(ln)
        w12 = np.zeros((ln, ln), np.float32)
        w3 = np.zeros((ln, ln), np.float32)
        w4 = np.zeros((ln, ln), np.float32)
        w12[loc, 2 * (loc // 2)] = 1.0
        w3[loc, 2 * (loc // 2) + 1] = 1.0
        w4[loc, 2 * (loc // 2) + 1] = -1.0
        t[f"w12_{u}"] = w12
        t[f"w3_{u}"] = w3
        t[f"w4_{u}"] = w4
    # w2 gather tables with conjugate folding
    w2r = np.zeros((ROWS, 82), np.float32)
    w2i = np.zeros((ROWS, 82), np.float32)
    for h in range(HDIM):
        pos, conj = REP_OF[h]
        sig = -1.0 if conj else 1.0
        for i in range(2):
            col = i * 41 + P + M_IDX[h]
            w2r[2 * pos, col] += fwm_wr[i, h]
            w2r[2 * pos + 1, col] += -sig * fwm_wi[i, h]
            w2i[2 * pos, col] += fwm_wi[i, h]
            w2i[2 * pos + 1, col] += sig * fwm_wr[i, h]
    for u, (o, ln) in enumerate(SUBCH):
        t[f"w2r_{u}"] = w2r[o:o + ln]
        t[f"w2i_{u}"] = w2i[o:o + ln]
    # r3 reduction tables.  q13 = str*[xr|xi] = [q1|q3]; q24 = sti*[xr|xi]
    # = [q4|q2]; Xa = [xr0*xr1|xi0*xi1] = [X1|X2]; Xb = [xr0*xi1|xi0*xr1]
    # = [X4|X3].
    w1z = conv1_w.copy(); w1z[P] = 0.0
    w2z = conv2_w.copy(); w2z[P] = 0.0
    taps = np.arange(41)
    for key, (qcol_off, qsign) in [("r3_q13a", (0, +0.5)),
                                   ("r3_q13b", (1, +0.5)),
                                   ("r3_q24a", (1, +0.5)),
                                   ("r3_q24b", (0, -0.5))]:
        r3 = np.zeros((82, F_ROWS), np.float32)
        for i in range(2):
            r3[i * 41 + taps, 2 * i + qcol_off] = qsign
        t[key] = r3
    for key, (xcol, xsign) in [("r3_xaa", (4, +0.5)), ("r3_xab", (4, +0.5)),
                               ("r3_xba", (5, -0.5)), ("r3_xbb", (5, +0.5))]:
        r3 = np.zeros((41, F_ROWS), np.float32)
        r3[taps, xcol] = xsign * w2z
        t[key] = r3
    pw = np.zeros((82, F_ROWS), np.float32)
    for i in range(2):
        for tap in range(41):
            r = i * 41 + tap
            pw[r, 6] += (2.0 if i == 0 else 1.0) * w1z[tap]
            pw[r, 7] += (2.0 if i == 1 else 1.0) * w1z[tap]
    for i in range(2):
        pw[i * 41 + P, 6] += 0.5 * C00
        pw[i * 41 + P, 7] += 0.5 * C00
    t["r3_pw"] = pw
    return t


def _pack_tables(tabs):
    import ml_dtypes
    bf = ml_dtypes.bfloat16
    cols = {}
    off = 0
    for k in sorted(tabs.keys()):
        v = tabs[k]
        cols[k] = (off, v.shape[0], v.shape[1])
        off += v.shape[1]
    buf = np.zeros((128, off), bf)
    for k, v in tabs.items():
        o, r, c = cols[k]
        buf[:r, o:o + c] = v.astype(bf)
    return buf, cols


_CACHED = {}


def _build_program(Bc, tab_cols, TC):
    import concourse.bacc as bacc
    import concourse.mybir as mybir
    import concourse.tile as tile

    f32 = mybir.dt.float32
    bf16 = mybir.dt.bfloat16
    Act = mybir.ActivationFunctionType
    Op = mybir.AluOpType
    NCHUNK = Bc // N            # 32
    NMB = NCHUNK // 2           # 16 megablocks (2 chunks each)
    MCOLS = Bc // 128           # 128

    nc = bacc.Bacc("TRN2", target_bir_lowering=False, debug=False,
                   num_devices=NCORES)

    # host-pregathered product operands: cols = (chunk, sub, half, 512)
    dXA = nc.dram_tensor("XA", [128, NCHUNK * 3 * 2 * N], bf16,
                         kind="ExternalInput").ap()
    dXC = nc.dram_tensor("XCP", [128, NCHUNK * 3 * 2 * N], bf16,
                         kind="ExternalInput").ap()
    # raw input tiles: XRI = [xr | xi] per chunk, XRI2 = mode-1 rows
    dXRI = nc.dram_tensor("XRI", [82, NCHUNK * 2 * N], bf16,
                          kind="ExternalInput").ap()
    dXRI2 = nc.dram_tensor("XRI2", [41, NCHUNK * 2 * N], bf16,
                           kind="ExternalInput").ap()
    dTAB = nc.dram_tensor("TAB", [128, TC], bf16, kind="ExternalInput").ap()
    dXCC = nc.dram_tensor("XCC", [128, 4 * MCOLS], f32,
                          kind="ExternalInput").ap()
    dT0 = nc.dram_tensor("T0M", [128, MCOLS], f32, kind="ExternalInput").ap()
    dID8 = nc.dram_tensor("ID8F", [F_ROWS, F_ROWS], f32,
                          kind="ExternalInput").ap()
    dOUT = nc.dram_tensor("OUT", [128, 4 * MCOLS], f32,
                          kind="ExternalOutput").ap()

    CPM = 3 * 2 * N             # product-operand cols per chunk (3072)
    RPM = 2 * N                 # XRI cols per chunk (1024)

    with tile.TileContext(nc) as tc:
        with (
            tc.tile_pool(name="consts", bufs=1) as cpool,
            tc.tile_pool(name="xa", bufs=3) as xapool,
            tc.tile_pool(name="xc", bufs=3) as xcpool,
            tc.tile_pool(name="xri", bufs=4) as xripool,
            tc.tile_pool(name="xri2", bufs=3) as xri2pool,
            tc.tile_pool(name="s12", bufs=5) as s12pool,
            tc.tile_pool(name="prod", bufs=3) as prodpool,
            tc.tile_pool(name="sa", bufs=4) as sapool,
            tc.tile_pool(name="st", bufs=3) as stpool,
            tc.tile_pool(name="mt", bufs=5) as mpool,
            tc.tile_pool(name="persist", bufs=1) as ppool,
            tc.tile_pool(name="fin", bufs=2) as fpool,
            tc.tile_pool(name="pas", bufs=3, space="PSUM") as pas,
            tc.tile_pool(name="pwr", bufs=2, space="PSUM") as pwr,
            tc.tile_pool(name="pwi", bufs=1, space="PSUM") as pwi,
            tc.tile_pool(name="pwf", bufs=2, space="PSUM") as pwf,
        ):
            # ---- PE warmup: keep the tensor engine continuously busy while
            # the first input DMAs land, so real matmuls start at full clock
            wtile = cpool.tile([F_ROWS, N], bf16, tag="wrm", name="wtile")
            nc.vector.memset(wtile[:], 0.0)
            wps = pwf.tile([F_ROWS, N], f32, tag="w", name="warm")
            for _ in range(20):
                nc.tensor.matmul(wps[:], wtile[0:F_ROWS, 0:F_ROWS], wtile[:],
                                 start=True, stop=True)

            # ---- constants (issued after the first chunk's data DMAs would
            # be nice, but table loads are small; keep simple) ----
            tabt = cpool.tile([128, TC], bf16, tag="tab", name="tabt")
            nc.scalar.dma_start(tabt[:], dTAB[:])

            def T(k):
                o, r, c = tab_cols[k]
                return tabt[0:r, o:o + c]

            t0m = cpool.tile([128, MCOLS], f32, tag="t0m", name="t0m")
            nc.scalar.dma_start(t0m[:], dT0[:])
            xcs = cpool.tile([128, 4 * MCOLS], f32, tag="xcs", name="xcs")
            nc.scalar.dma_start(xcs[:], dXCC[:])
            ident8f = cpool.tile([F_ROWS, F_ROWS], f32, tag="id8f",
                                 name="ident8f")
            nc.scalar.dma_start(ident8f[:], dID8[:])

            Mt = ppool.tile([128, NCHUNK * 32], f32, tag="mega", name="mega")
            OUTs = ppool.tile([128, 4 * MCOLS], f32, tag="outs", name="outs")

            pend = []
            qpend = []
            w2pend = []
            fpend = []

            def emit_chain2(state):
                Ms, s12t, c = state
                q13t, q24t, xat, xbt = Ms
                pF = pwf.tile([F_ROWS, N], f32, tag="w", name="pF")
                nc.tensor.matmul(pF[:], T("r3_pw"), s12t[:, 0:N],
                                 start=True, stop=False)
                nc.tensor.matmul(pF[:], T("r3_pw"), s12t[:, N:2 * N],
                                 start=False, stop=False)
                for tbl, tile_, half in [("r3_q13a", q13t, 0),
                                         ("r3_q13b", q13t, 1),
                                         ("r3_q24a", q24t, 0),
                                         ("r3_q24b", q24t, 1),
                                         ("r3_xaa", xat, 0),
                                         ("r3_xab", xat, 1),
                                         ("r3_xba", xbt, 0),
                                         ("r3_xbb", xbt, 1)]:
                    rows = tab_cols[tbl][1]
                    nc.tensor.matmul(pF[:], T(tbl),
                                     tile_[0:rows, half * N:(half + 1) * N],
                                     start=False, stop=(tbl == "r3_xbb"))
                fpend.append((pF, c))

            def emit_chain3():
                pF, c = fpend.pop(0)
                sF = fpool.tile([F_ROWS, N], f32, tag="sF", name="sF")
                nc.scalar.activation(sF[:], pF[:], Act.Copy)
                pG = pwf.tile([128, 32], f32, tag="w", name="pG")
                for tq in range(4):
                    nc.tensor.transpose(pG[:, tq * 8:tq * 8 + 8],
                                        sF[:, tq * 128:(tq + 1) * 128],
                                        ident8f[:])
                nc.scalar.activation(Mt[:, c * 32:(c + 1) * 32], pG[:],
                                     Act.Copy)

            def emit_mb(mb):
                XAt = xapool.tile([128, 2 * CPM], bf16, tag="xa", name="XAt")
                XCt = xcpool.tile([128, 2 * CPM], bf16, tag="xc", name="XCt")
                XRIt = xripool.tile([82, 2 * RPM], bf16, tag="xri",
                                    name="XRIt")
                XRI2t = xri2pool.tile([41, 2 * RPM], bf16, tag="xri2",
                                      name="XRI2t")
                for ch in range(2):
                    c = 2 * mb + ch
                    hs_ = slice(ch * CPM, (ch + 1) * CPM)
                    gs_ = slice(c * CPM, (c + 1) * CPM)
                    nc.sync.dma_start(XAt[:, hs_], dXA[:, gs_])
                    nc.sync.dma_start(XCt[:, hs_], dXC[:, gs_])
                    if ch == 0:
                        nc.sync.dma_start(
                            XRIt[:], dXRI[:, mb * 2 * RPM:(mb + 1) * 2 * RPM])
                        nc.sync.dma_start(
                            XRI2t[:],
                            dXRI2[:, mb * 2 * RPM:(mb + 1) * 2 * RPM])
                    xac = XAt[:, ch * CPM:(ch + 1) * CPM]
                    xcc = XCt[:, ch * CPM:(ch + 1) * CPM]
                    xric = XRIt[:, ch * RPM:(ch + 1) * RPM]
                    xri2c = XRI2t[:, ch * RPM:(ch + 1) * RPM]

                    # |x|^2 -> s12 = [xr^2 | xi^2] (Pool)
                    s12t = s12pool.tile([82, 2 * N], bf16, tag="s12",
                                        name="s12")
                    nc.gpsimd.tensor_tensor(s12t[:], xric, xric, Op.mult)

                    # ---- FWM products + As accumulation per sub-chunk ----
                    sas = [None, None, None]
                    for u, (o, ln) in enumerate(SUBCH):
                        cu = u * 2 * N
                        xau = xac[0:ln, cu:cu + 2 * N]
                        xcu = xcc[0:ln, cu:cu + 2 * N]
                        p12 = prodpool.tile([128, 2 * N], bf16, tag="p12",
                                            name="p12")
                        p34 = prodpool.tile([128, 2 * N], bf16, tag="p34",
                                            name="p34")
                        nc.vector.tensor_tensor(p12[0:ln, :], xau, xcu,
                                                Op.mult)
                        rev = xau.rearrange("p (k n) -> p k n",
                                            k=2)[:, ::-1, :]
                        nc.vector.tensor_tensor(
                            p34[0:ln, :].rearrange("p (k n) -> p k n", k=2),
                            rev, xcu.rearrange("p (k n) -> p k n", k=2),
                            Op.mult)
                        pAs = pas.tile([128, N], f32, tag="as", name="pAs")
                        nc.tensor.matmul(pAs[0:ln, :], T(f"w12_{u}"),
                                         p12[0:ln, 0:N],
                                         start=True, stop=False)
                        nc.tensor.matmul(pAs[0:ln, :], T(f"w12_{u}"),
                                         p12[0:ln, N:2 * N],
                                         start=False, stop=False)
                        nc.tensor.matmul(pAs[0:ln, :], T(f"w3_{u}"),
                                         p34[0:ln, 0:N],
                                         start=False, stop=False)
                        nc.tensor.matmul(pAs[0:ln, :], T(f"w4_{u}"),
                                         p34[0:ln, N:2 * N],
                                         start=False, stop=True)
                        sa_u = sapool.tile([128, N], bf16, tag=f"sa{u}",
                                           name=f"sa{u}")
                        nc.scalar.activation(sa_u[0:ln, :], pAs[0:ln, :],
                                             Act.Copy)
                        sas[u] = sa_u

                    # chain tail (sF/transpose/Mt) for chunk c-2: its pF
                    # finished last chunk, so Act never stalls on it
                    if fpend:
                        emit_chain3()

                    # conv2 cross products need only DMA inputs -> same chunk
                    xat = mpool.tile([41, 2 * N], bf16, tag="xa2", name="xat")
                    xbt = mpool.tile([41, 2 * N], bf16, tag="xb2", name="xbt")
                    nc.vector.tensor_tensor(xat[:], xric[0:41, :], xri2c,
                                            Op.mult)
                    rev2 = xri2c.rearrange("p (k n) -> p k n", k=2)[:, ::-1, :]
                    nc.vector.tensor_tensor(
                        xbt[:].rearrange("p (k n) -> p k n", k=2),
                        xric[0:41, :].rearrange("p (k n) -> p k n", k=2),
                        rev2, Op.mult)
                    # ---- q products for the PREVIOUS chunk (str/sti ready,
                    # so they never head-block the DVE/Pool queues) ----
                    if len(qpend) > 1:
                        pstr, psti, pxric, ps12, pxat, pxbt, pc = qpend.pop(0)
                        q13 = mpool.tile([82, 2 * N], bf16, tag="q13",
                                         name="q13")
                        q24 = mpool.tile([82, 2 * N], bf16, tag="q24",
                                         name="q24")
                        nc.vector.tensor_tensor(
                            q13[:].rearrange("p (k n) -> p k n", k=2),
                            pstr[:].unsqueeze(1).broadcast_to([82, 2, N]),
                            pxric.rearrange("p (k n) -> p k n", k=2),
                            Op.mult)
                        nc.gpsimd.tensor_tensor(
                            q24[:].rearrange("p (k n) -> p k n", k=2),
                            psti[:].unsqueeze(1).broadcast_to([82, 2, N]),
                            pxric.rearrange("p (k n) -> p k n", k=2),
                            Op.mult)
                        pend.append(((q13, q24, pxat, pxbt), ps12, pc))

                    # previous chunk's reduction: after w2 so it never gates
                    # the next chunk's PE block
                    if len(pend) > 0:
                        emit_chain2(pend.pop(0))
                    w2pend.append((sas, xric, s12t, xat, xbt, c))
                    if len(w2pend) > 1:
                        psas, w_xric, w_s12, w_xat, w_xbt, wc = w2pend.pop(0)
                        # ---- w2 gather for chunk c-1 (inputs a chunk old,
                        # so PE never waits on the sa copies) ----
                        ptr = pwr.tile([82, N], f32, tag="wr", name="ptr")
                        for u, (o, ln) in enumerate(SUBCH):
                            nc.tensor.matmul(ptr[:], T(f"w2r_{u}"),
                                             psas[u][0:ln, :], start=(u == 0),
                                             stop=(u == 2))
                        strt = stpool.tile([82, N], bf16, tag="str",
                                           name="strt")
                        nc.scalar.activation(strt[:], ptr[:], Act.Copy)
                        pti = pwi.tile([82, N], f32, tag="wi", name="pti")
                        for u, (o, ln) in enumerate(SUBCH):
                            nc.tensor.matmul(pti[:], T(f"w2i_{u}"),
                                             psas[u][0:ln, :], start=(u == 0),
                                             stop=(u == 2))
                        stit = stpool.tile([82, N], bf16, tag="sti",
                                           name="stit")
                        nc.scalar.activation(stit[:], pti[:], Act.Copy)
                        qpend.append((strt, stit, w_xric, w_s12, w_xat,
                                      w_xbt, wc))
                    # final combine pieces, pipelined into chunk slack:
                    # group g (chunks 8g..8g+7) is ready after chain2(8g+7),
                    # emitted during chunk 8g+8; prelude at 8g+10, one combo
                    # per chunk at 8g+11..8g+14
                    if c >= 11 and (c - 11) % 8 == 0:
                        emit_final_pre((c - 11) // 8)
                    if c >= 12 and (c - 12) % 8 <= 3:
                        gg = (c - 12) // 8
                        emit_final_combo(gg, (c - 12) % 8)

            fstate = {}

            def emit_final_pre(g):
                """Prelude for final group g (chunks 8g..8g+7): power + trig."""
                GC = 32
                gs = slice(GC * g, GC * (g + 1))
                Pht = fpool.tile([128, GC], f32, tag="fA", name="Pht")
                LN10_10 = float(np.log(10.0) / 10.0)
                nc.scalar.activation(Pht[:], t0m[:, gs], Act.Exp,
                                     scale=LN10_10)
                Mtv = Mt[:].rearrange("p (g k) -> p g k", k=8)

                def msl(k):
                    return Mtv[:, gs, k]

                phi0 = fpool.tile([128, GC], f32, tag="fB", name="phi0")
                phi1 = fpool.tile([128, GC], f32, tag="fB", name="phi1")
                nc.vector.tensor_tensor(phi0[:], Pht[:], msl(6), Op.mult)
                nc.vector.tensor_tensor(phi1[:], Pht[:], msl(7), Op.mult)
                c0 = fpool.tile([128, GC], f32, tag="fC", name="c0")
                s0 = fpool.tile([128, GC], f32, tag="fC", name="s0")
                c1 = fpool.tile([128, GC], f32, tag="fC", name="c1")
                s1_ = fpool.tile([128, GC], f32, tag="fC", name="s1")
                nc.scalar.activation(c0[:], phi0[:], Act.Sin, bias=hpi[:])
                nc.scalar.activation(s0[:], phi0[:], Act.Sin)
                nc.scalar.activation(c1[:], phi1[:], Act.Sin, bias=hpi[:])
                nc.scalar.activation(s1_[:], phi1[:], Act.Sin)
                fstate[g] = (Pht, {"c0": c0, "s0": s0, "c1": c1, "s1": s1_})

            FCOMBOS = [
                (0, [(2, 5, -1.0), (3, 4, -1.0)], (0, "c0", +1.0),
                 (1, "s0", -1.0), 0),
                (1, [(2, 4, +1.0), (3, 5, -1.0)], (0, "s0", +1.0),
                 (1, "c0", +1.0), 1),
                (2, [(0, 5, +1.0), (1, 4, -1.0)], (2, "c1", +1.0),
                 (3, "s1", -1.0), 2),
                (3, [(0, 4, +1.0), (1, 5, +1.0)], (2, "s1", +1.0),
                 (3, "c1", +1.0), 3),
            ]

            def emit_final_combo(g, i):
                GC = 32
                gs = slice(GC * g, GC * (g + 1))
                Pht, trig = fstate[g]
                Mtv = Mt[:].rearrange("p (g k) -> p g k", k=8)

                def msl(k):
                    return Mtv[:, gs, k]

                def xcb(q):
                    return xcs[:, q * MCOLS + GC * g:q * MCOLS + GC * (g + 1)]

                fidx, prodl, term1, term2, outq = FCOMBOS[i]
                acc = fpool.tile([128, GC], f32, tag="fD", name="acc")
                nc.vector.tensor_copy(acc[:], msl(fidx))
                for (ka, kb, sgn) in prodl:
                    tmp = fpool.tile([128, GC], f32, tag="fE", name="tmp")
                    nc.vector.tensor_tensor(tmp[:], xcb(ka), msl(kb), Op.mult)
                    nc.vector.tensor_tensor(
                        acc[:], acc[:], tmp[:],
                        Op.add if sgn > 0 else Op.subtract)
                nc.vector.tensor_tensor(acc[:], acc[:], Pht[:], Op.mult)
                for (kc, tkey, sgn) in (term1, term2):
                    tmp = fpool.tile([128, GC], f32, tag="fE", name="tmp")
                    nc.vector.tensor_tensor(tmp[:], xcb(kc), trig[tkey][:],
                                            Op.mult)
                    nc.vector.tensor_tensor(
                        acc[:], acc[:], tmp[:],
                        Op.add if sgn > 0 else Op.subtract)
                nc.vector.tensor_copy(
                    OUTs[:, outq * MCOLS + GC * g:
                         outq * MCOLS + GC * (g + 1)], acc[:])
                if i == 3:
                    for q in range(4):
                        nc.sync.dma_start(
                            dOUT[:, q * MCOLS + GC * g:
                                 q * MCOLS + GC * (g + 1)],
                            OUTs[:, q * MCOLS + GC * g:
                                 q * MCOLS + GC * (g + 1)])

            hpi = cpool.tile([128, 1], f32, tag="hpi", name="hpi")
            nc.vector.memset(hpi[:], float(np.pi / 2))

            for mb in range(NMB):
                emit_mb(mb)

            # drain: w2 for the last chunk, then q products + reductions
            while w2pend:
                psas, w_xric, w_s12, w_xat, w_xbt, wc = w2pend.pop(0)
                ptr = pwr.tile([82, N], f32, tag="wr", name="ptr")
                for u, (o, ln) in enumerate(SUBCH):
                    nc.tensor.matmul(ptr[:], T(f"w2r_{u}"), psas[u][0:ln, :],
                                     start=(u == 0), stop=(u == 2))
                strt = stpool.tile([82, N], bf16, tag="str", name="strt")
                nc.scalar.activation(strt[:], ptr[:], Act.Copy)
                pti = pwi.tile([82, N], f32, tag="wi", name="pti")
                for u, (o, ln) in enumerate(SUBCH):
                    nc.tensor.matmul(pti[:], T(f"w2i_{u}"), psas[u][0:ln, :],
                                     start=(u == 0), stop=(u == 2))
                stit = stpool.tile([82, N], bf16, tag="sti", name="stit")
                nc.scalar.activation(stit[:], pti[:], Act.Copy)
                qpend.append((strt, stit, w_xric, w_s12, w_xat, w_xbt, wc))
            while qpend:
                pstr, psti, pxric, ps12, pxat, pxbt, pc = qpend.pop(0)
                q13 = mpool.tile([82, 2 * N], bf16, tag="q13", name="q13")
                q24 = mpool.tile([82, 2 * N], bf16, tag="q24", name="q24")
                nc.vector.tensor_tensor(
                    q13[:].rearrange("p (k n) -> p k n", k=2),
                    pstr[:].unsqueeze(1).broadcast_to([82, 2, N]),
                    pxric.rearrange("p (k n) -> p k n", k=2), Op.mult)
                nc.gpsimd.tensor_tensor(
                    q24[:].rearrange("p (k n) -> p k n", k=2),
                    psti[:].unsqueeze(1).broadcast_to([82, 2, N]),
                    pxric.rearrange("p (k n) -> p k n", k=2), Op.mult)
                pend.append(((q13, q24, pxat, pxbt), ps12, pc))
                if fpend:
                    emit_chain3()
                emit_chain2(pend.pop(0))
            while fpend:
                emit_chain3()
            emit_final_pre(3)
            for i in range(4):
                emit_final_combo(3, i)

    nc.compile()
    return nc


def kernel(**inputs):
    from concourse.bass_utils import run_bass_kernel_spmd

    trace = bool(inputs.pop("_trace", False))
    x_real = np.asarray(inputs["x_real"], dtype=np.float32)
    x_imag = np.asarray(inputs["x_imag"], dtype=np.float32)
    task_info = np.asarray(inputs["task_info"], dtype=np.float32)
    C00 = float(np.asarray(inputs["C00"]).reshape(-1)[0])
    fwm_wr = np.asarray(inputs["fwm_wr"], dtype=np.float32)
    fwm_wi = np.asarray(inputs["fwm_wi"], dtype=np.float32)
    conv1_w = np.asarray(inputs["conv1_w"], dtype=np.float32)
    conv2_w = np.asarray(inputs["conv2_w"], dtype=np.float32)

    B = x_real.shape[0]
    Bc = B // NCORES
    NCHUNK = Bc // N

    tabs = _build_tables(fwm_wr, fwm_wi, conv1_w, conv2_w, C00)
    tab_buf, tab_cols = _pack_tables(tabs)
    TC = tab_buf.shape[1]

    if "nc" not in _CACHED:
        _CACHED["nc"] = _build_program(Bc, tab_cols, TC)
    nc = _CACHED["nc"]

    import ml_dtypes
    bf = ml_dtypes.bfloat16
    in_maps = []
    mcols = Bc // 128
    for core in range(NCORES):
        sl = slice(core * Bc, (core + 1) * Bc)
        xr = np.ascontiguousarray(
            x_real[sl].transpose(2, 1, 0).reshape(82, Bc)).astype(bf)
        xi = np.ascontiguousarray(
            x_imag[sl].transpose(2, 1, 0).reshape(82, Bc)).astype(bf)

        # pre-gathered FWM product operands [128, (chunk, sub, half, 512)]
        XA = np.zeros((128, NCHUNK, 3, 2, N), bf)
        XC = np.zeros((128, NCHUNK, 3, 2, N), bf)
        for u, (o, ln) in enumerate(SUBCH):
            ra = SRC_A[o:o + ln]
            rc = SRC_C[o:o + ln]
            XA[0:ln, :, u, 0, :] = xr[ra].reshape(ln, NCHUNK, N)
            XA[0:ln, :, u, 1, :] = xi[ra].reshape(ln, NCHUNK, N)
            XC[0:ln, :, u, 0, :] = xr[rc].reshape(ln, NCHUNK, N)
            XC[0:ln, :, u, 1, :] = xi[rc].reshape(ln, NCHUNK, N)

        # raw tiles: XRI = [xr | xi] per chunk; XRI2 = mode-1 rows
        XRI = np.zeros((82, NCHUNK, 2, N), bf)
        XRI[:, :, 0, :] = xr.reshape(82, NCHUNK, N)
        XRI[:, :, 1, :] = xi.reshape(82, NCHUNK, N)
        XRI2 = np.zeros((41, NCHUNK, 2, N), bf)
        XRI2[:, :, 0, :] = xr[41:82].reshape(41, NCHUNK, N)
        XRI2[:, :, 1, :] = xi[41:82].reshape(41, NCHUNK, N)

        t0 = task_info[sl, 0]
        T0M = np.ascontiguousarray(
            t0.reshape(Bc // 512, 4, 128).transpose(2, 0, 1).reshape(
                128, Bc // 128))
        XCC = np.empty((128, 4 * mcols), np.float32)
        for qi, arr in enumerate([x_real[sl, P, 0], x_imag[sl, P, 0],
                                  x_real[sl, P, 1], x_imag[sl, P, 1]]):
            XCC[:, qi * mcols:(qi + 1) * mcols] = np.ascontiguousarray(
                arr.reshape(Bc // 512, 4, 128).transpose(2, 0, 1).reshape(
                    128, mcols))
        m = {"XA": XA.reshape(128, -1), "XCP": XC.reshape(128, -1),
             "XRI": XRI.reshape(82, -1), "XRI2": XRI2.reshape(41, -1),
             "T0M": T0M, "XCC": XCC, "TAB": tab_buf,
             "ID8F": np.eye(F_ROWS, dtype=np.float32)}
        in_maps.append(m)

    res = run_bass_kernel_spmd(nc, in_maps, list(range(NCORES)), trace=trace)
    _CACHED["last_exec_ns"] = res.exec_time_ns

    outs = []
    cols = Bc // 128
    for core in range(NCORES):
        OUT = res.results[core]["OUT"]
        E = np.empty((Bc, 2), np.complex64)
        for q, (dst, im) in enumerate([(0, 0), (0, 1), (1, 0), (1, 1)]):
            O = OUT[:, q * cols:(q + 1) * cols]
            flat = np.ascontiguousarray(
                O.reshape(128, Bc // 512, 4).transpose(1, 2, 0)).reshape(Bc)
            if im == 0:
                E[:, dst] = flat
            else:
                E[:, dst] += 1j * flat.astype(np.complex64)
        outs.append(E)
    return np.concatenate(outs, axis=0)
